# revision 1
# baseline (speedup 1.0000x reference)
"""Local (causal, windowed) attention block on 8 trn2 NeuronCores.

Sharding: sequence-parallel. 8 shards = batch(2) x seq-quarter(4); each core
computes 512 output tokens and needs a 256-token K/V halo on the left. All
matmuls run as float32r (full-rate fp32) on the tensor engine; data flows in
transposed [feature, token] layout so Q/K/scores/attn-out chain without
intermediate transposes. Softmax denominators come from an appended ones
column in the V operand; masking is additive (host-precomputed band masks).
"""

import numpy as np

import concourse.bass as bass  # noqa: F401  (engine types referenced via nc)
import concourse.mybir as mybir
import concourse.tile as tile
from concourse import bacc
from concourse.bass_utils import run_bass_kernel_spmd
from concourse.masks import make_identity

B, S, D = 2, 2048, 1024
H, DH = 16, 64
WIN = 256
TOK, HALO = 512, 256
XT = TOK + HALO  # 768
F32 = mybir.dt.float32
F32R = mybir.dt.float32r

# per-key-chunk query windows (qlo, qwidth); kc=3 widened to 512 so the
# first attn@V matmul (start=True) covers the whole psum bank.
KC_WIN = [(0, 256), (0, 256), (0, 384), (0, 512), (256, 256), (256, 256)]
KC_ORDER = [3, 0, 1, 2, 4, 5]

_cache = {}


def r(ap):
    return ap.bitcast(F32R)


def build_nc():
    nc = bacc.Bacc(None, target_bir_lowering=False)
    xh_d = nc.declare_dram_parameter("xh", [XT, D], F32, isOutput=False)
    msk_d = nc.declare_dram_parameter("msk", [6, 128, 512], F32, isOutput=False)
    wq_d = nc.declare_dram_parameter("wq", [D, D], F32R, isOutput=False)
    wk_d = nc.declare_dram_parameter("wk", [D, D], F32R, isOutput=False)
    wv_d = nc.declare_dram_parameter("wv", [D, D], F32R, isOutput=False)
    wo_d = nc.declare_dram_parameter("wo", [D, D], F32R, isOutput=False)
    bq_d = nc.declare_dram_parameter("bq", [D], F32, isOutput=False)  # pre-scaled 1/8
    bk_d = nc.declare_dram_parameter("bk", [D], F32, isOutput=False)
    bv_d = nc.declare_dram_parameter("bv", [D], F32R, isOutput=False)
    bo_d = nc.declare_dram_parameter("bo", [D], F32R, isOutput=False)
    ones_d = nc.declare_dram_parameter("ones", [128, 128], F32R, isOutput=False)
    out_d = nc.declare_dram_parameter("out", [TOK, D], F32, isOutput=True)

    Exp = mybir.ActivationFunctionType.Exp
    Ident = mybir.ActivationFunctionType.Identity

    with tile.TileContext(nc) as tc:
        with (
            tc.tile_pool(name="const", bufs=1) as const,
            tc.tile_pool(name="persist", bufs=1) as persist,
            tc.tile_pool(name="w", bufs=9) as wpool,
        ):
            ident = const.tile([128, 128], F32)
            make_identity(nc, ident)
            ones_col = const.tile([1, 128], F32R)
            nc.sync.dma_start(out=ones_col[:], in_=ones_d[0:1, :])
            # per-outcol-chunk bias columns: [:, 0:8]=bq/8, [:, 8:16]=bk
            bqk = const.tile([128, 16], F32)
            nc.sync.dma_start(out=bqk[:, 0:8], in_=bq_d[:].rearrange("(c p) -> p c", p=128))
            nc.sync.dma_start(out=bqk[:, 8:16], in_=bk_d[:].rearrange("(c p) -> p c", p=128))
            bv_sb = const.tile([1, D], F32R)
            nc.sync.dma_start(out=bv_sb[:], in_=bv_d[:].rearrange("(a d) -> a d", a=1))
            bo_sb = const.tile([1, D], F32R)
            nc.sync.dma_start(out=bo_sb[:], in_=bo_d[:].rearrange("(a d) -> a d", a=1))
            mask_sb = []
            for kc in range(6):
                mt = const.tile([128, 512], F32, name=f"mask{kc}")
                nc.sync.dma_start(out=mt[:], in_=msk_d[kc])
                mask_sb.append(mt)

            QT = [persist.tile([128, TOK], F32R, name=f"QT{i}") for i in range(8)]
            KT = [persist.tile([128, XT], F32R, name=f"KT{i}") for i in range(8)]
            Vg = [persist.tile([128, 16 * 65], F32R, name=f"Vg{i}") for i in range(6)]
            AO = [persist.tile([128, TOK], F32R, name=f"AO{i}") for i in range(8)]

            # ---- Phase A: load x, transpose to xT [feat, tok] ----
            with tc.tile_pool(name="xTp", bufs=1) as xTp:
                xTt = [xTp.tile([128, XT], F32R, name=f"xT{i}") for i in range(8)]
                with (
                    tc.tile_pool(name="xload", bufs=3) as xpool,
                    tc.tile_pool(name="tpsum", bufs=4, space="PSUM") as tpsum,
                ):
                    for tt in range(6):
                        xt = xpool.tile([128, D], F32, tag="xt")
                        nc.sync.dma_start(out=xt[:], in_=xh_d[tt * 128:(tt + 1) * 128, :])
                        for fc in range(8):
                            pt = tpsum.tile([128, 128], F32, tag="tp")
                            nc.tensor.transpose(pt[:], xt[:, fc * 128:(fc + 1) * 128], ident[:])
                            nc.vector.tensor_copy(xTt[fc][:, tt * 128:(tt + 1) * 128], pt[:])

                # ---- Phase B: QT = (wq.T @ xT)/8 + bq/8 ; KT = wk.T @ xT + bk ----
                wq_sb = []
                for kc in range(8):
                    wt = wpool.tile([128, D], F32R, tag="w", name=f"wq{kc}")
                    nc.sync.dma_start(out=wt[:], in_=wq_d[kc * 128:(kc + 1) * 128, :])
                    wq_sb.append(wt)
                wk_sb = []
                for kc in range(8):
                    wt = wpool.tile([128, D], F32R, tag="w", name=f"wk{kc}")
                    nc.sync.dma_start(out=wt[:], in_=wk_d[kc * 128:(kc + 1) * 128, :])
                    wk_sb.append(wt)

                with tc.tile_pool(name="qpsum", bufs=4, space="PSUM") as qpsum:
                    for oc in range(8):
                        ps = qpsum.tile([128, TOK], F32, tag="pp")
                        for kc in range(8):
                            nc.tensor.matmul(
                                ps[:],
                                lhsT=(wq_sb[kc][:, oc * 128:(oc + 1) * 128]),
                                rhs=(xTt[kc][:, HALO:XT]),
                                start=(kc == 0), stop=(kc == 7),
                            )
                        # QT pre-scaled by 1/8 (bias arrives pre-scaled from host)
                        nc.scalar.activation(QT[oc][:], ps[:], Ident,
                                             bias=bqk[:, oc:oc + 1], scale=0.125)
                    for oc in range(8):
                        for hf in range(2):
                            ps = qpsum.tile([128, 384], F32, tag="pp", padded_shape=[128, 512])
                            for kc in range(8):
                                nc.tensor.matmul(
                                    ps[:],
                                    lhsT=(wk_sb[kc][:, oc * 128:(oc + 1) * 128]),
                                    rhs=(xTt[kc][:, hf * 384:(hf + 1) * 384]),
                                    start=(kc == 0), stop=(kc == 7),
                                )
                            nc.scalar.activation(KT[oc][:, hf * 384:(hf + 1) * 384],
                                                 ps[:], Ident,
                                                 bias=bqk[:, 8 + oc:9 + oc], scale=1.0)

                    # ---- Phase C: V (natural layout) + ones column -> Vg ----
                    wv_sb = []
                    for kc in range(8):
                        wt = wpool.tile([128, D], F32R, tag="w", name=f"wv{kc}")
                        nc.sync.dma_start(out=wt[:], in_=wv_d[kc * 128:(kc + 1) * 128, :])
                        wv_sb.append(wt)
                    for tt in range(6):
                        for hf in range(2):
                            ps = qpsum.tile([128, 512], F32, tag="pp")
                            nc.tensor.matmul(ps[:], lhsT=(ones_col[:]),
                                             rhs=(bv_sb[:, hf * 512:(hf + 1) * 512]),
                                             start=True, stop=False)
                            for kc in range(8):
                                nc.tensor.matmul(
                                    ps[:],
                                    lhsT=(xTt[kc][:, tt * 128:(tt + 1) * 128]),
                                    rhs=(wv_sb[kc][:, hf * 512:(hf + 1) * 512]),
                                    start=False, stop=(kc == 7),
                                )
                            dst = Vg[tt][:, hf * 520:(hf + 1) * 520].rearrange(
                                "p (h d) -> p h d", d=65)[:, :, 0:64]
                            nc.scalar.copy(dst, ps[:].rearrange("p (h d) -> p h d", d=64))
                        nc.sync.dma_start(
                            out=Vg[tt][:].rearrange("p (h d) -> p h d", d=65)[:, :, 64:65],
                            in_=ones_d[:, 0:16])

            # ---- Phase D: attention ----
            # prefetch wo while attention runs
            wo_sb = []
            for kc in range(8):
                wt = wpool.tile([128, D], F32R, tag="w", name=f"wo{kc}")
                nc.sync.dma_start(out=wt[:], in_=wo_d[kc * 128:(kc + 1) * 128, :])
                wo_sb.append(wt)

            with (
                tc.tile_pool(name="spsum", bufs=4, space="PSUM") as spsum,
                tc.tile_pool(name="opsum", bufs=3, space="PSUM") as opsum,
                tc.tile_pool(name="bpsum", bufs=1, space="PSUM") as bpsum,
                tc.tile_pool(name="es", bufs=10) as es_pool,
                tc.tile_pool(name="sadd", bufs=5) as sadd_pool,
                tc.tile_pool(name="rec", bufs=3) as rec_pool,
                tc.tile_pool(name="bc", bufs=2) as bc_pool,
            ):
                for h in range(H):
                    g, ho = h // 2, (h % 2) * 64
                    es_tiles = {}
                    for kc in KC_ORDER:
                        qlo, qw = KC_WIN[kc]
                        ps = spsum.tile([128, qw], F32, tag="sp", padded_shape=[128, 512])
                        nc.tensor.matmul(
                            ps[:],
                            lhsT=(KT[g][ho:ho + 64, kc * 128:(kc + 1) * 128]),
                            rhs=(QT[g][ho:ho + 64, qlo:qlo + qw]),
                            start=True, stop=True,
                        )
                        sa = sadd_pool.tile([128, qw], F32, tag="sa", padded_shape=[128, 512])
                        nc.vector.tensor_add(sa[:], ps[:], mask_sb[kc][:, qlo:qlo + qw])
                        es = es_pool.tile([128, qw], F32R, tag="es", padded_shape=[128, 512])
                        nc.scalar.activation(es[:], sa[:], Exp)
                        es_tiles[kc] = es
                    po = opsum.tile([65, TOK], F32, tag="op")
                    for i, kc in enumerate(KC_ORDER):
                        qlo, qw = KC_WIN[kc]
                        nc.tensor.matmul(
                            po[:, qlo:qlo + qw],
                            lhsT=(Vg[kc][:, h * 65:(h + 1) * 65]),
                            rhs=(es_tiles[kc][:]),
                            start=(i == 0), stop=(i == 5),
                        )
                    den = rec_pool.tile([1, TOK], F32, tag="den")
                    rsc = rec_pool.tile([1, TOK], F32, tag="rsc")
                    rec = rec_pool.tile([1, TOK], F32R, tag="rec")
                    nc.scalar.copy(den[:], po[64:65, :])
                    nc.vector.reciprocal_approx_fast(rsc[:], den[:])
                    with nc.allow_low_precision(reason="fp32r feed to bcast matmul"):
                        nc.vector.tensor_copy(rec[:], rsc[:])
                    pb = bpsum.tile([64, TOK], F32, tag="bp")
                    nc.tensor.matmul(pb[:], lhsT=(ones_col[:, 0:64]), rhs=(rec[:]),
                                     start=True, stop=True)
                    bc = bc_pool.tile([64, TOK], F32, tag="bc")
                    nc.scalar.copy(bc[:], pb[:])
                    nc.vector.tensor_mul(AO[g][ho:ho + 64, :], po[0:64, :], bc[:])

            # ---- Phase E: out = AO.T @ wo + bo ----
            with (
                tc.tile_pool(name="fpsum", bufs=3, space="PSUM") as fpsum,
                tc.tile_pool(name="oout", bufs=3) as oout,
            ):
                for tt in range(4):
                    ot = oout.tile([128, D], F32, tag="oo")
                    for hf in range(2):
                        ps = fpsum.tile([128, 512], F32, tag="fp")
                        nc.tensor.matmul(ps[:], lhsT=(ones_col[:]),
                                         rhs=(bo_sb[:, hf * 512:(hf + 1) * 512]),
                                         start=True, stop=False)
                        for kc in range(8):
                            nc.tensor.matmul(
                                ps[:],
                                lhsT=(AO[kc][:, tt * 128:(tt + 1) * 128]),
                                rhs=(wo_sb[kc][:, hf * 512:(hf + 1) * 512]),
                                start=False, stop=(kc == 7),
                            )
                        nc.vector.tensor_copy(ot[:, hf * 512:(hf + 1) * 512], ps[:])
                    nc.sync.dma_start(out=out_d[tt * 128:(tt + 1) * 128, :], in_=ot[:])

    nc.compile()
    return nc


def _mask_for_chunk(c):
    m = np.full((6, 128, 512), -30000.0, np.float32)
    for kc in range(6):
        k = kc * 128 + np.arange(128)[:, None]
        q = np.arange(512)[None, :]
        valid = (q >= k - WIN) & (q <= k)
        if c == 0:
            valid = valid & (k >= HALO)
        m[kc][valid] = 0.0
    return m


def _round_f32r(a):
    """Round fp32 values to the fp32r grid (11-bit mantissa, RNE): the PE
    consumes fp32r operands, i.e. fp32 words with the low 12 bits zeroed."""
    u = np.ascontiguousarray(a, np.float32).view(np.uint32)
    tie = (u >> 12) & np.uint32(1)
    u2 = (u + np.uint32(0x7FF) + tie) & np.uint32(0xFFFFF000)
    return u2.view(np.float32)


def kernel(x, wq, bq, wk, bk, wv, bv, wo, bo):
    x = np.ascontiguousarray(np.asarray(x, np.float32))
    wq = _round_f32r(np.asarray(wq, np.float32))
    wk = _round_f32r(np.asarray(wk, np.float32))
    wv = _round_f32r(np.asarray(wv, np.float32))
    wo = _round_f32r(np.asarray(wo, np.float32))
    bq8 = np.ascontiguousarray(np.asarray(bq, np.float32) * 0.125)
    bk = np.ascontiguousarray(np.asarray(bk, np.float32))
    bv = _round_f32r(np.asarray(bv, np.float32))
    bo = _round_f32r(np.asarray(bo, np.float32))

    if "nc" not in _cache:
        _cache["nc"] = build_nc()
        _cache["masks"] = [_mask_for_chunk(c) for c in range(4)]
    nc = _cache["nc"]
    masks = _cache["masks"]
    ones_c = np.ones((128, 128), np.float32)

    in_maps = []
    for core in range(8):
        b, c = divmod(core, 4)
        start = c * TOK
        xh = np.zeros((XT, D), np.float32)
        lo = max(0, start - HALO)
        xh[HALO - (start - lo):] = x[b, lo:start + TOK]
        in_maps.append({
            "xh": xh, "msk": masks[c],
            "wq": wq, "wk": wk, "wv": wv, "wo": wo,
            "bq": bq8, "bk": bk, "bv": bv, "bo": bo,
            "ones": ones_c,
        })
    _cache["last_in_maps"] = in_maps
    res = run_bass_kernel_spmd(nc, in_maps, list(range(8)))
    out = np.empty((B, S, D), np.float32)
    for core in range(8):
        b, c = divmod(core, 4)
        out[b, c * TOK:(c + 1) * TOK] = res.results[core]["out"]
    return out



# revision 11
# speedup vs baseline: 1.2788x; 1.2788x over previous
"""Local (causal, windowed) attention block on 8 trn2 NeuronCores.

Sharding: sequence-parallel. 8 shards = batch(2) x seq-quarter(4); each core
computes 512 output tokens and needs a 256-token K/V halo on the left.

All matmul operands are bf16 (psum accumulation stays f32): same 1 cycle/row
PE rate as fp32r but half the DMA/SBUF traffic and less PE power (fp32r at
full rate trips the DVFS utilization throttle). Data flows in transposed
[feature, token] layout so Q/K/scores/attn-out chain without transposes.
Softmax denominators come from an appended ones column in the V operand;
masking is additive (host-precomputed band masks, applied in-place in PSUM,
alternating DVE/Pool). bv and bo fold host-side into bo_eff = bv@wo + bo
(softmax rows sum to 1), added during the phase-E psum->sbuf copy.
The attention head loop is software-pipelined: scores(h) are emitted before
attn@V(h-1) so the PE never waits on the softmax chain; the 1/den broadcast
runs once per head pair via a 2-row selector matmul.
"""

import numpy as np
import ml_dtypes

import concourse.bass as bass  # noqa: F401
import concourse.mybir as mybir
import concourse.tile as tile
from concourse import bacc
from concourse.bass_utils import run_bass_kernel_spmd

B, S, D = 2, 2048, 1024
H, DH = 16, 64
WIN = 256
TOK, HALO = 512, 256
XT = TOK + HALO  # 768
F32 = mybir.dt.float32
F32R = mybir.dt.float32r
BF16 = mybir.dt.bfloat16

# per-key-chunk query windows (qlo, qwidth); kc=3 widened to 512 so the
# first attn@V matmul (start=True) covers the whole psum bank.
KC_WIN = [(0, 128), (0, 256), (0, 384), (0, 512), (256, 256), (384, 128)]
KC_ORDER = [3, 0, 1, 2, 4, 5]

_cache = {}


def r(ap):
    return ap.bitcast(F32R)


def build_nc():
    nc = bacc.Bacc(None, target_bir_lowering=False)
    xh_d = nc.declare_dram_parameter("xh", [XT, D], BF16, isOutput=False)
    msk_d = nc.declare_dram_parameter("msk", [6, 128, 512], BF16, isOutput=False)
    wq_d = nc.declare_dram_parameter("wq", [D, D], BF16, isOutput=False)
    wk_d = nc.declare_dram_parameter("wk", [D, D], BF16, isOutput=False)
    wv_d = nc.declare_dram_parameter("wv", [D, D], BF16, isOutput=False)
    wo_d = nc.declare_dram_parameter("wo", [D, D], BF16, isOutput=False)
    bq_d = nc.declare_dram_parameter("bq", [D], F32, isOutput=False)  # pre-scaled 1/8
    bk_d = nc.declare_dram_parameter("bk", [D], F32, isOutput=False)
    e2_d = nc.declare_dram_parameter("e2", [33, 128], BF16, isOutput=False)
    ones16_d = nc.declare_dram_parameter("ones16", [128, 16], BF16, isOutput=False)
    ident_d = nc.declare_dram_parameter("ident", [128, 128], BF16, isOutput=False)
    z33_d = nc.declare_dram_parameter("z33", [33, TOK], BF16, isOutput=False)
    bob_d = nc.declare_dram_parameter("bob", [128, D], F32, isOutput=False)
    out_d = nc.declare_dram_parameter("out", [TOK, D], F32, isOutput=True)

    Exp = mybir.ActivationFunctionType.Exp
    Ident = mybir.ActivationFunctionType.Identity

    with tile.TileContext(nc) as tc:
        with (
            tc.tile_pool(name="const", bufs=1) as const,
            tc.tile_pool(name="persist", bufs=1) as persist,
            tc.tile_pool(name="w", bufs=9) as wpool,
        ):
            ident = const.tile([128, 128], BF16)
            nc.sync.dma_start(out=ident[:], in_=ident_d[:, :])
            e2 = const.tile([33, 128], BF16)
            nc.sync.dma_start(out=e2[:], in_=e2_d[:, :])
            # per-outcol-chunk bias columns: [:, 0:8]=bq/8, [:, 8:16]=bk
            bqk = const.tile([128, 16], F32)
            nc.sync.dma_start(out=bqk[:, 0:8], in_=bq_d[:].rearrange("(c p) -> p c", p=128))
            nc.sync.dma_start(out=bqk[:, 8:16], in_=bk_d[:].rearrange("(c p) -> p c", p=128))
            bob = const.tile([128, D], F32)
            nc.sync.dma_start(out=bob[:], in_=bob_d[:, :])
            mask_sb = []
            for kc in range(6):
                mt = const.tile([128, 512], BF16, name=f"mask{kc}")
                nc.sync.dma_start(out=mt[:], in_=msk_d[kc])
                mask_sb.append(mt)

            # 1/den staging for head pairs: rows 0/32 are written per pair,
            # rows 1-31 stay zero (DMA-initialized; HW SBUF powers up as garbage)
            rcb_t = [persist.tile([33, TOK], BF16, name=f"rcb{i}") for i in range(2)]
            nc.sync.dma_start(out=rcb_t[0][:], in_=z33_d[:, :])
            nc.sync.dma_start(out=rcb_t[1][:], in_=z33_d[:, :])
            QT = [persist.tile([128, TOK], BF16, name=f"QT{i}") for i in range(8)]
            KT = [persist.tile([128, XT], BF16, name=f"KT{i}") for i in range(8)]
            Vg = [persist.tile([128, 16 * 65], BF16, name=f"Vg{i}") for i in range(6)]
            AO = [persist.tile([128, TOK], BF16, name=f"AO{i}") for i in range(8)]

            # ---- Phase A: load x (bf16), transpose to xT [feat, tok] ----
            with tc.tile_pool(name="xTp", bufs=1) as xTp:
                xTt = [xTp.tile([128, XT], BF16, name=f"xT{i}") for i in range(8)]
                with (
                    tc.tile_pool(name="xload", bufs=3) as xpool,
                    tc.tile_pool(name="tpsum", bufs=4, space="PSUM") as tpsum,
                ):
                    for tt in range(6):
                        xt = xpool.tile([128, D], BF16, tag="xt")
                        nc.sync.dma_start(out=xt[:], in_=xh_d[tt * 128:(tt + 1) * 128, :])
                        for fc in range(8):
                            pt = tpsum.tile([128, 128], BF16, tag="tp")
                            nc.tensor.transpose(pt[:], xt[:, fc * 128:(fc + 1) * 128], ident[:])
                            if (tt * 8 + fc) % 2 == 0:
                                nc.vector.tensor_copy(xTt[fc][:, tt * 128:(tt + 1) * 128], pt[:])
                            else:
                                nc.scalar.copy(xTt[fc][:, tt * 128:(tt + 1) * 128], pt[:])

                # ---- Phase B: QT = (wq.T @ xT)/8 + bq/8 ; KT = wk.T @ xT + bk ----
                wq_sb = []
                for kc in range(8):
                    wt = wpool.tile([128, D], BF16, tag="w", name=f"wq{kc}")
                    nc.sync.dma_start(out=wt[:], in_=wq_d[kc * 128:(kc + 1) * 128, :])
                    wq_sb.append(wt)
                wk_sb = []
                for kc in range(8):
                    wt = wpool.tile([128, D], BF16, tag="w", name=f"wk{kc}")
                    nc.sync.dma_start(out=wt[:], in_=wk_d[kc * 128:(kc + 1) * 128, :])
                    wk_sb.append(wt)

                with tc.tile_pool(name="qpsum", bufs=4, space="PSUM") as qpsum:
                    for oc in range(8):
                        ps = qpsum.tile([128, TOK], F32, tag="pp")
                        for kc in range(8):
                            nc.tensor.matmul(
                                ps[:],
                                lhsT=wq_sb[kc][:, oc * 128:(oc + 1) * 128],
                                rhs=xTt[kc][:, HALO:XT],
                                start=(kc == 0), stop=(kc == 7),
                            )
                        # QT pre-scaled by 1/8 (bias arrives pre-scaled from host)
                        nc.scalar.activation(QT[oc][:], ps[:], Ident,
                                             bias=bqk[:, oc:oc + 1], scale=0.125)
                    for oc in range(8):
                        for hf in range(2):
                            ps = qpsum.tile([128, 384], F32, tag="pp", padded_shape=[128, 512])
                            for kc in range(8):
                                nc.tensor.matmul(
                                    ps[:],
                                    lhsT=wk_sb[kc][:, oc * 128:(oc + 1) * 128],
                                    rhs=xTt[kc][:, hf * 384:(hf + 1) * 384],
                                    start=(kc == 0), stop=(kc == 7),
                                )
                            nc.scalar.activation(KT[oc][:, hf * 384:(hf + 1) * 384],
                                                 ps[:], Ident,
                                                 bias=bqk[:, 8 + oc:9 + oc], scale=1.0)

                    # ---- Phase C: V (natural layout, no bias) + ones column ----
                    wv_sb = []
                    for kc in range(8):
                        wt = wpool.tile([128, D], BF16, tag="w", name=f"wv{kc}")
                        nc.sync.dma_start(out=wt[:], in_=wv_d[kc * 128:(kc + 1) * 128, :])
                        wv_sb.append(wt)
                    for tt in range(6):
                        for hf in range(2):
                            ps = qpsum.tile([128, 512], F32, tag="pp")
                            for kc in range(8):
                                nc.tensor.matmul(
                                    ps[:],
                                    lhsT=xTt[kc][:, tt * 128:(tt + 1) * 128],
                                    rhs=wv_sb[kc][:, hf * 512:(hf + 1) * 512],
                                    start=(kc == 0), stop=(kc == 7),
                                )
                            dst = Vg[tt][:, hf * 520:(hf + 1) * 520].rearrange(
                                "p (h d) -> p h d", d=65)[:, :, 0:64]
                            nc.scalar.copy(dst, ps[:].rearrange("p (h d) -> p h d", d=64))
                        nc.sync.dma_start(
                            out=Vg[tt][:].rearrange("p (h d) -> p h d", d=65)[:, :, 64:65],
                            in_=ones16_d[:, 0:16])

            # ---- Phase D: attention (head loop software-pipelined by 1) ----
            # prefetch wo while attention runs
            wo_sb = []
            for kc in range(8):
                wt = wpool.tile([128, D], BF16, tag="w", name=f"wo{kc}")
                nc.sync.dma_start(out=wt[:], in_=wo_d[kc * 128:(kc + 1) * 128, :])
                wo_sb.append(wt)

            with (
                tc.tile_pool(name="spsum", bufs=4, space="PSUM") as spsum,
                tc.tile_pool(name="opsum", bufs=3, space="PSUM") as opsum,
                tc.tile_pool(name="bpsum", bufs=1, space="PSUM") as bpsum,
                tc.tile_pool(name="es", bufs=14) as es_pool,
                tc.tile_pool(name="bc", bufs=2) as bc_pool,
                tc.tile_pool(name="den", bufs=3) as den_pool,
            ):
                es_all = [None] * H
                po_all = [None] * H

                def emit_scores(h):
                    g, ho = h // 2, (h % 2) * 64
                    es_tiles = {}
                    for j, kc in enumerate(KC_ORDER):
                        qlo, qw = KC_WIN[kc]
                        ps = spsum.tile([128, qw], F32, tag="sp", padded_shape=[128, 512])
                        nc.tensor.matmul(
                            ps[:],
                            lhsT=KT[g][ho:ho + 64, kc * 128:(kc + 1) * 128],
                            rhs=QT[g][ho:ho + 64, qlo:qlo + qw],
                            start=True, stop=True,
                        )
                        es = es_pool.tile([128, qw], BF16, tag="es", padded_shape=[128, 512])
                        nc.scalar.activation(es[:], ps[:], Exp)
                        with nc.allow_low_precision(reason="bf16 es mask"):
                            nc.gpsimd.tensor_mul(es[:], es[:], mask_sb[kc][:, qlo:qlo + qw])
                        es_tiles[kc] = es
                    es_all[h] = es_tiles

                def emit_av(h):
                    g, ho = h // 2, (h % 2) * 64
                    es_tiles = es_all[h]
                    po = opsum.tile([65, TOK], F32, tag="op")
                    for i, kc in enumerate(KC_ORDER):
                        qlo, qw = KC_WIN[kc]
                        nc.tensor.matmul(
                            po[:, qlo:qlo + qw],
                            lhsT=Vg[kc][:, h * 65:(h + 1) * 65],
                            rhs=es_tiles[kc][:],
                            start=(i == 0), stop=(i == 5),
                        )
                    po_all[h] = po
                    # denominator reciprocal straight from psum row 64; the
                    # bf16 copy lands on partition 0/32 of the pair tile
                    # (DVE/Pool writes must start on a 32-aligned partition)
                    h2 = h % 2
                    rcb = rcb_t[(h // 2) % 2]
                    den = den_pool.tile([1, TOK], F32, tag="dn")
                    nc.scalar.copy(den[:], po[64:65, :])
                    rsc = den_pool.tile([1, TOK], F32, tag="rs")
                    nc.vector.reciprocal_approx_fast(rsc[:], den[:])
                    with nc.allow_low_precision(reason="bf16 1/den"):
                        nc.vector.tensor_copy(rcb[32 * h2:32 * h2 + 1, :], rsc[:])
                    if h2 == 1:
                        # one broadcast matmul per head pair: pb rows 0:64 get
                        # 1/den of head h-1, rows 64:128 get head h's
                        pb = bpsum.tile([128, TOK], F32, tag="bp")
                        nc.tensor.matmul(pb[:], lhsT=e2[:], rhs=rcb[:],
                                         start=True, stop=True)
                        bc = bc_pool.tile([128, TOK], F32, tag="bc")
                        nc.scalar.copy(bc[:], pb[:])
                        with nc.allow_low_precision(reason="bf16 attn output"):
                            nc.vector.tensor_mul(AO[g][0:64, :], po_all[h - 1][0:64, :],
                                                 bc[0:64, :])
                            nc.vector.tensor_mul(AO[g][64:128, :], po[0:64, :],
                                                 bc[64:128, :])

                for i in range(H + 1):
                    if i < H:
                        emit_scores(i)
                    if i >= 1:
                        emit_av(i - 1)

            # ---- Phase E: out = AO.T @ wo + bo_eff ----
            with (
                tc.tile_pool(name="fpsum", bufs=3, space="PSUM") as fpsum,
                tc.tile_pool(name="oout", bufs=3) as oout,
            ):
                for tt in range(4):
                    ot = oout.tile([128, D], F32, tag="oo")
                    for hf in range(2):
                        ps = fpsum.tile([128, 512], F32, tag="fp")
                        for kc in range(8):
                            nc.tensor.matmul(
                                ps[:],
                                lhsT=AO[kc][:, tt * 128:(tt + 1) * 128],
                                rhs=wo_sb[kc][:, hf * 512:(hf + 1) * 512],
                                start=(kc == 0), stop=(kc == 7),
                            )
                        nc.vector.tensor_add(ot[:, hf * 512:(hf + 1) * 512], ps[:],
                                                 bob[:, hf * 512:(hf + 1) * 512])
                    nc.sync.dma_start(out=out_d[tt * 128:(tt + 1) * 128, :], in_=ot[:])

    nc.compile()
    return nc


def _mask_for_chunk(c):
    m = np.zeros((6, 128, 512), np.float32)
    for kc in range(6):
        k = kc * 128 + np.arange(128)[:, None]
        q = np.arange(512)[None, :]
        valid = (q >= k - WIN) & (q <= k)
        if c == 0:
            valid = valid & (k >= HALO)
        m[kc][valid] = 1.0
    return m.astype(ml_dtypes.bfloat16)


def kernel(x, wq, bq, wk, bk, wv, bv, wo, bo):
    bf = ml_dtypes.bfloat16
    x = np.asarray(x, np.float32)
    wq16 = np.ascontiguousarray(np.asarray(wq, np.float32).astype(bf))
    wk16 = np.ascontiguousarray(np.asarray(wk, np.float32).astype(bf))
    wv16 = np.ascontiguousarray(np.asarray(wv, np.float32).astype(bf))
    wo32 = np.asarray(wo, np.float32)
    wo16 = np.ascontiguousarray(wo32.astype(bf))
    bq8 = np.ascontiguousarray(np.asarray(bq, np.float32) * 0.125)
    bk = np.ascontiguousarray(np.asarray(bk, np.float32))
    # fold bv through wo (softmax rows sum to 1): out += bv @ wo + bo
    bo_eff = (np.asarray(bv, np.float32) @ wo32 + np.asarray(bo, np.float32)).astype(np.float32)
    bob = np.ascontiguousarray(np.broadcast_to(bo_eff, (128, D)))
    e2 = np.zeros((33, 128), np.float32).astype(bf)
    e2[0, 0:64] = 1.0
    e2[32, 64:128] = 1.0
    ones16 = np.ones((128, 16), bf)
    ident = np.eye(128, dtype=np.float32).astype(bf)
    z33 = np.zeros((33, TOK), bf)

    if "nc" not in _cache:
        _cache["nc"] = build_nc()
        _cache["masks"] = [_mask_for_chunk(c) for c in range(4)]
    nc = _cache["nc"]
    masks = _cache["masks"]

    in_maps = []
    for core in range(8):
        b, c = divmod(core, 4)
        start = c * TOK
        xh = np.zeros((XT, D), np.float32)
        lo = max(0, start - HALO)
        xh[HALO - (start - lo):] = x[b, lo:start + TOK]
        in_maps.append({
            "xh": np.ascontiguousarray(xh.astype(bf)), "msk": masks[c],
            "wq": wq16, "wk": wk16, "wv": wv16, "wo": wo16,
            "bq": bq8, "bk": bk, "e2": e2, "ones16": ones16,
            "ident": ident, "bob": bob, "z33": z33,
        })
    _cache["last_in_maps"] = in_maps
    res = run_bass_kernel_spmd(nc, in_maps, list(range(8)))
    out = np.empty((B, S, D), np.float32)
    for core in range(8):
        b, c = divmod(core, 4)
        out[b, c * TOK:(c + 1) * TOK] = res.results[core]["out"]
    return out
